# revision 49
# baseline (speedup 1.0000x reference)
"""AdaptiveLinearWithChannel: per-channel complex matmul with hypernet rank-2
residual, sharded channel-parallel across 8 TRN2 NeuronCores.

out[c] = x[c] @ (W[model_idx,c] + u_c v_c^T) + bias[model_idx,c] + hyper_shift[c]
  x: (C=32, P=8192, D=128) complex; W_eff: (C, D, D) complex.

Host: tiny hypernet MLPs (1->10->10->{8D,2D}) + rank-2 residual -> W_eff and
combined shift (float64), then cast x to bf16 and pre-transpose to (C, D, P)
so the device needs no on-chip transposes and half the DMA bytes. The matmul
consumes bf16 either way, so the input cast adds no error vs casting on-chip.

Device (per core, 4 channels): for each 128-row chunk of x, two accumulating
bf16 matmuls with stationary xT chunks and column-interleaved moving operands
(Wr_0,Wi_0,Wr_1,...) and (-Wi_0,Wr_0,-Wi_1,...), N=256 -> psum holds the
complex-interleaved output directly; the epilogue is a single DVE tensor_add
per 4-chunk batch (adds bias + hyper_shift, writes bf16 to SBUF), stores use
a partition-major DRAM layout for 16KB-contiguous DMA runs. Host widens
bf16 -> complex64 and restores row order. DMA-bound: ~34MB/core at
~390 GB/s ~= 87us stream + ~11us fixed NEFF overhead.
"""

import sys

sys.path.insert(0, "/opt/trn_rl_repo")

import numpy as np

C, P, D = 32, 8192, 128
N_CORES = 8
CH = C // N_CORES  # channels per core
PSUB = 4096        # p-columns per DMA slab
NCHUNK = PSUB // 128
NJ = P // PSUB     # slabs per channel
NB = 4             # 128-chunks batched per PSUM tile / epilogue op

_NC_CACHE = {}


def _build_nc():
    from concourse import bacc, mybir
    from concourse.tile import TileContext

    f32 = mybir.dt.float32
    bf16 = mybir.dt.bfloat16

    nc = bacc.Bacc()
    # x_real/x_imag combined: (c, d, 0, p)=re, (c, d, 1, p)=im -> one DMA/slab
    xt = nc.declare_dram_parameter("xt", [CH, D, 2, P], bf16, isOutput=False)
    # both pre-arranged on host to the exact SBUF layout (partition-major);
    # only [Wr|Wi] ships — [-Wi|Wr] is derived on-device
    wmov = nc.declare_dram_parameter("wmov", [D, CH, 2 * D], bf16, isOutput=False)
    shift = nc.declare_dram_parameter("shift", [D, CH, 2 * D], bf16, isOutput=False)
    # partition-major output layout: 16KB contiguous runs per partition on the
    # store DMA; host transposes (j, p128, k) -> rows afterwards.
    out = nc.declare_dram_parameter(
        "out", [CH, NJ, 128, NCHUNK, 2 * D], bf16, isOutput=True
    )

    with TileContext(nc) as tc:
        with (
            tc.tile_pool(name="const", bufs=1) as cpool,
            tc.tile_pool(name="xin", bufs=5) as xpool,
            tc.tile_pool(name="pop", bufs=4, space="PSUM") as popool,
            tc.tile_pool(name="oout", bufs=3) as opool,
        ):
            # prologue params on the scalar HWDGE queue (idle at start; the
            # sync queue begins streaming x slabs immediately)
            w_bf = cpool.tile([128, CH, 2 * D], bf16, tag="wbf")
            nc.scalar.dma_start(out=w_bf[:], in_=wmov[:])
            # wmov columns are host-interleaved (Wr_0, Wi_0, Wr_1, Wi_1, ...)
            # so psum accumulates the complex-interleaved output directly.
            # Derive the conjugate-partner operand (-Wi_0, Wr_0, ...) on DVE.
            w_neg = cpool.tile([128, CH, 2 * D], bf16, tag="wneg")
            nc.vector.tensor_scalar_mul(
                w_neg[:, :, 0 : 2 * D : 2], w_bf[:, :, 1 : 2 * D : 2], -1.0
            )
            nc.vector.tensor_copy(
                w_neg[:, :, 1 : 2 * D : 2], w_bf[:, :, 0 : 2 * D : 2]
            )

            # shift tiles: [p, c, 2D] bf16 (host pre-broadcast across partitions)
            shift_sb = cpool.tile([128, CH, 2 * D], bf16, tag="shift")
            nc.scalar.dma_start(out=shift_sb[:], in_=shift[:])

            for c in range(CH):
                w_r_slice = w_bf[:, c, :]
                w_i_slice = w_neg[:, c, :]
                for j in range(NJ):
                    x_slab = xpool.tile([128, 2, PSUB], bf16, tag="xri")
                    nc.sync.dma_start(
                        out=x_slab[:], in_=xt[c, :, :, j * PSUB : (j + 1) * PSUB]
                    )
                    out_sb = opool.tile([128, NCHUNK, 2 * D], bf16, tag="osb")
                    q = NCHUNK // 2
                    for k0 in range(0, NCHUNK, NB):
                        if k0 % q == 0 and k0 > 0:
                            # store finished quarter while the rest computes
                            nc.scalar.dma_start(
                                out=out[c, j, :, k0 - q : k0, :],
                                in_=out_sb[:, k0 - q : k0, :],
                            )
                        po = popool.tile([128, NB, 2 * D], f32, tag="po")
                        for b in range(NB):
                            k = k0 + b
                            nc.tensor.matmul(
                                po[:, b, :],
                                x_slab[:, 0, k * 128 : (k + 1) * 128],
                                w_r_slice,
                                start=True,
                                stop=False,
                            )
                            nc.tensor.matmul(
                                po[:, b, :],
                                x_slab[:, 1, k * 128 : (k + 1) * 128],
                                w_i_slice,
                                start=False,
                                stop=True,
                            )
                        # epilogue: psum already complex-interleaved; one add
                        # applies the (interleaved) shift and moves to SBUF
                        nc.vector.tensor_add(
                            out_sb[:, k0 : k0 + NB, :],
                            po[:, :, :],
                            shift_sb[:, c : c + 1, :].broadcast_to(
                                [128, NB, 2 * D]
                            ),
                        )
                    nc.scalar.dma_start(
                        out=out[c, j, :, NCHUNK - q : NCHUNK, :],
                        in_=out_sb[:, NCHUNK - q : NCHUNK, :],
                    )
    nc.compile()
    return nc


def _host_prep(inputs):
    """Hypernet MLPs + rank-2 residual on host (float64), -> per-core arrays."""
    import ml_dtypes

    bf16 = ml_dtypes.bfloat16

    def relu(a):
        return np.maximum(a, 0.0)

    t = np.asarray(inputs["t"], np.float64)  # (1, 1)
    idx = np.asarray(inputs["indices"])

    def hyper(W1, b1, W2, b2, W3, b3):
        W1, b1, W2, b2, W3, b3 = (
            np.asarray(p, np.float64)[idx] for p in (W1, b1, W2, b2, W3, b3)
        )
        h = relu(np.einsum("ti,cio->cto", t, W1) + b1[:, None, :])
        h = relu(np.einsum("cti,cio->cto", h, W2) + b2[:, None, :])
        return np.einsum("cti,cio->cto", h, W3) + b3[:, None, :]

    uv = hyper(*(inputs[k] for k in ("gW1", "gb1", "gW2", "gb2", "gW3", "gb3")))
    uv = uv[:, 0, :]  # (C, 8D)  (nt == 1)
    u = (uv[:, : 2 * D] + 1j * uv[:, 2 * D : 4 * D]).reshape(C, D, 2)
    v = (uv[:, 4 * D : 6 * D] + 1j * uv[:, 6 * D :]).reshape(C, D, 2)
    residual = u @ np.swapaxes(v, -1, -2)  # (C, D, D)

    mi = int(np.asarray(inputs["model_idx"]))
    weight = np.asarray(inputs["weight"], np.float64)
    bias = np.asarray(inputs["bias"], np.float64)
    w = weight[mi, ..., 0] + 1j * weight[mi, ..., 1]  # (C, D, D)
    b = bias[mi, ..., 0] + 1j * bias[mi, ..., 1]  # (C, 1, D)

    W_eff = w + residual  # (C, D, D)

    hs = hyper(*(inputs[k] for k in ("sW1", "sb1", "sW2", "sb2", "sW3", "sb3")))
    hs = hs[:, 0, :]  # (C, 2D)
    shift = b[:, 0, :] + (hs[:, :D] + 1j * hs[:, D:])  # (C, D)

    Wr = W_eff.real.astype(np.float32)
    Wi = W_eff.imag.astype(np.float32)

    # moving operand with interleaved columns (Wr_0, Wi_0, Wr_1, Wi_1, ...),
    # stored partition(d)-major to match SBUF layout
    wmov = np.empty((C, D, 2 * D), np.float32)
    wmov[:, :, 0::2] = Wr
    wmov[:, :, 1::2] = Wi
    wmov = wmov.astype(bf16)  # (C, D, 2D)

    # shift tile, interleaved and broadcast across the 128 partitions
    shift_t = np.empty((C, D, 2 * D), np.float32)
    shift_t[:, :, 0::2] = shift.real.astype(np.float32)[:, None, :]
    shift_t[:, :, 1::2] = shift.imag.astype(np.float32)[:, None, :]
    shift_t = shift_t.astype(bf16)

    # x: cast to bf16 (RNE) and transpose to (C, D, 2, P) — device needs no
    # on-chip transposes, and real/imag combine into one DMA per slab
    xt = np.empty((C, D, 2, P), bf16)
    xt[:, :, 0, :] = np.asarray(inputs["x_real"], np.float32).transpose(0, 2, 1)
    xt[:, :, 1, :] = np.asarray(inputs["x_imag"], np.float32).transpose(0, 2, 1)

    in_maps = []
    for core in range(N_CORES):
        c0 = core * CH
        in_maps.append(
            {
                "xt": xt[c0 : c0 + CH],
                # (CH,D,2D) -> (D,CH,2D)
                "wmov": np.ascontiguousarray(
                    wmov[c0 : c0 + CH].transpose(1, 0, 2)
                ),
                "shift": np.ascontiguousarray(
                    shift_t[c0 : c0 + CH].transpose(1, 0, 2)
                ),
            }
        )
    return in_maps


def _assemble(outs):
    """bf16 (CH, NJ, 128, NCHUNK, 2D) per core -> (1, C, P, D) complex64."""
    full = np.concatenate(outs, axis=0)  # (C, NJ, 128, NCHUNK, 2D) bf16
    full = full.transpose(0, 1, 3, 2, 4).reshape(C, P, 2 * D)
    u32 = full.view(np.uint16).astype(np.uint32) << 16
    f32 = u32.view(np.float32)
    return np.ascontiguousarray(f32).view(np.complex64)[None]


def _get_nc():
    if "nc" not in _NC_CACHE:
        _NC_CACHE["nc"] = _build_nc()
    return _NC_CACHE["nc"]


def kernel(**inputs):
    from concourse.bass_utils import run_bass_kernel_spmd

    nc = _get_nc()
    in_maps = _host_prep(inputs)
    res = run_bass_kernel_spmd(nc, in_maps, core_ids=list(range(N_CORES)))
    return _assemble([res.results[i]["out"] for i in range(N_CORES)])
